# revision 10
# baseline (speedup 1.0000x reference)
"""GQA attention block (B=2,T=2048,E=2048,H=16,KV=4) on 8 trn2 NeuronCores.

Sharding: core c -> batch b=c//4, kv-group g=c%4 (q-heads 4g..4g+3, kv head g).
Each core computes its 4 heads end-to-end plus the partial output projection
(Wo rows for its heads); host sums the 4 partials per batch and adds bias.

v3 schedule/layout:
  - Chunk-pipelined: per 512-token chunk c, emit attention(qc=c), K/V
    projections of c+1, out-proj(c), Q projections of c+1 — PE never idles
    at phase boundaries, HAM stays warm.
  - RoPE half-swap via a permutation matmul on PE (no SBUF->SBUF DMA, which
    serialized against the vA transpose-DMAs).
  - Causal diagonal processed at 128-column granularity (skips the upper
    triangle's wasted matmul/exp work); single [128,128] triangle mask.
  - Softmax denominators: exp tiles accumulated in bf16 on DVE (2x mode),
    one all-ones[128,128] matmul broadcasts column sums to every partition,
    reciprocal_approx_fast + tensor_mul divides.
  - V projection computes V^T (wv stationary -> LDWEIGHTS hidden), XBAR
    transpose-DMA into [s,d] tiles for the PV matmul.
  - Output stored f16; host accumulates partials in f64.
"""

import numpy as np

for _p in ("/opt/trn_rl_repo", "/root/.axon_site/_ro/trn_rl_repo"):
    import sys

    if _p not in sys.path:
        sys.path.insert(0, _p)

import ml_dtypes
from contextlib import ExitStack

import concourse.bass as bass
import concourse.mybir as mybir
import concourse.tile as tile
from concourse import bacc
from concourse.bass_utils import run_bass_kernel_spmd

F32 = mybir.dt.float32
BF16 = mybir.dt.bfloat16
F16 = mybir.dt.float16
T = 2048
E = 2048
HD = 128
NQH = 4          # q heads per core
KT = E // 128    # 16 k-tiles over embed
NC = T // 512    # 4 512-chunks over time
SCALE = float(E) ** -0.5

_program = None
LAST_EXEC_NS = None
LAST_TRACE = None
LAST_PROFILE_JSON = None


def _build_program():
    nc = bacc.Bacc("TRN2", target_bir_lowering=False, debug=False, num_devices=8)
    xT_d = nc.declare_dram_parameter("xT", [E, T], F16, isOutput=False)
    wq_d = nc.declare_dram_parameter("wq", [E, NQH * HD], F16, isOutput=False)
    wk_d = nc.declare_dram_parameter("wk", [E, HD], F16, isOutput=False)
    wv_d = nc.declare_dram_parameter("wv", [E, HD], F16, isOutput=False)
    wo_d = nc.declare_dram_parameter("wo", [NQH * HD, E], BF16, isOutput=False)
    ct_d = nc.declare_dram_parameter("ct", [HD, T], F32, isOutput=False)
    st_d = nc.declare_dram_parameter("st", [HD, T], F32, isOutput=False)
    tri_d = nc.declare_dram_parameter("tri", [HD, HD], BF16, isOutput=False)
    psw_d = nc.declare_dram_parameter("psw", [HD, HD], BF16, isOutput=False)
    bq_d = nc.declare_dram_parameter("bq", [HD, NQH], F32, isOutput=False)
    bk_d = nc.declare_dram_parameter("bk", [HD, 1], F32, isOutput=False)
    out_d = nc.declare_dram_parameter("out", [T, E], F16, isOutput=True)

    with tile.TileContext(nc) as tc, ExitStack() as ctx:
        consts = ctx.enter_context(tc.tile_pool(name="consts", bufs=1))
        rope = ctx.enter_context(tc.tile_pool(name="rope", bufs=2))
        vsp = ctx.enter_context(tc.tile_pool(name="vsp", bufs=2))
        ptp = ctx.enter_context(tc.tile_pool(name="ptp", bufs=6))
        accp = ctx.enter_context(tc.tile_pool(name="accp", bufs=2))
        accbp = ctx.enter_context(tc.tile_pool(name="accbp", bufs=2))
        rinvp = ctx.enter_context(tc.tile_pool(name="rinvp", bufs=2))
        otp = ctx.enter_context(tc.tile_pool(name="otp", bufs=6))
        outp = ctx.enter_context(tc.tile_pool(name="outp", bufs=2))
        psP = ctx.enter_context(tc.tile_pool(name="psP", bufs=2, space=bass.MemorySpace.PSUM))
        psS = ctx.enter_context(tc.tile_pool(name="psS", bufs=2, space=bass.MemorySpace.PSUM))
        psW = ctx.enter_context(tc.tile_pool(name="psW", bufs=1, space=bass.MemorySpace.PSUM))
        psOT = ctx.enter_context(tc.tile_pool(name="psOT", bufs=2, space=bass.MemorySpace.PSUM))
        psB = ctx.enter_context(tc.tile_pool(name="psB", bufs=1, space=bass.MemorySpace.PSUM))

        # ---- persistent tiles ---------------------------------------------
        wk = consts.tile([128, KT * HD], F16, tag="wk", name="wk")
        wv = consts.tile([128, KT * HD], F16, tag="wv", name="wv")
        wq = consts.tile([128, KT * NQH * HD], F16, tag="wq", name="wq")
        wo = consts.tile([128, NQH * E], BF16, tag="wo", name="wo")
        xtc = [consts.tile([128, KT * 512], F16, tag=f"xtc{c}", name=f"xtc{c}")
               for c in range(NC)]
        ctc = [consts.tile([128, 512], F32, tag=f"ctc{c}", name=f"ctc{c}")
               for c in range(NC)]
        stc = [consts.tile([128, 512], F32, tag=f"stc{c}", name=f"stc{c}")
               for c in range(NC)]
        tri = consts.tile([128, 128], BF16, tag="tri", name="tri")
        psw = consts.tile([128, 128], BF16, tag="psw", name="psw")
        bq_t = consts.tile([HD, NQH], F32, tag="bq", name="bq_t")
        bk_t = consts.tile([HD, 1], F32, tag="bk", name="bk_t")
        ones128 = consts.tile([128, 128], BF16, tag="ones", name="ones128")
        nc.vector.memset(ones128[:], 1.0)

        qTc = [[consts.tile([128, 512], BF16, tag=f"qT{h}_{c}", name=f"qT{h}_{c}")
                for c in range(NC)] for h in range(NQH)]
        kTc = [consts.tile([128, 512], BF16, tag=f"kT{c}", name=f"kT{c}")
               for c in range(NC)]
        vA = [consts.tile([128, 128], BF16, tag=f"vA{t}", name=f"vA{t}")
              for t in range(4 * NC)]

        def split_rows(src_ap, p=128):
            # [(k p), f] -> [p, k, f]: one DMA that deposits each 128-row
            # band k into its own column block of the destination tile.
            return src_ap.rearrange("(k p) f -> p k f", p=p)

        # ---- input DMA issue order (chunk-major so compute starts early) --
        nc.sync.dma_start(wk[:], split_rows(wk_d[:, :]))
        nc.sync.dma_start(bk_t[:], bk_d[:, :])
        nc.sync.dma_start(bq_t[:], bq_d[:, :])
        nc.sync.dma_start(psw[:], psw_d[:, :])
        nc.sync.dma_start(tri[:], tri_d[:, :])
        nc.sync.dma_start(xtc[0][:], split_rows(xT_d[:, 0:512]))
        nc.sync.dma_start(wv[:], split_rows(wv_d[:, :]))
        nc.sync.dma_start(wq[:], split_rows(wq_d[:, :]))
        nc.sync.dma_start(ctc[0][:], ct_d[:, 0:512])
        nc.sync.dma_start(stc[0][:], st_d[:, 0:512])
        for c in range(1, NC):
            nc.sync.dma_start(xtc[c][:], split_rows(xT_d[:, c * 512:(c + 1) * 512]))
            nc.sync.dma_start(ctc[c][:], ct_d[:, c * 512:(c + 1) * 512])
            nc.sync.dma_start(stc[c][:], st_d[:, c * 512:(c + 1) * 512])
            if c == 2:
                nc.sync.dma_start(wo[:], split_rows(wo_d[:, :]))

        # ---- projections ---------------------------------------------------
        def rope_chunk(ps, bias_ap, dst, c):
            qsb = rope.tile([128, 512], BF16, tag="qsb", name="qsb")
            nc.scalar.activation(
                qsb[:], ps[:], mybir.ActivationFunctionType.Identity, bias=bias_ap)
            qsw = psW.tile([128, 512], F32, tag="qsw", name="qsw")
            nc.tensor.matmul(qsw[:], psw[:], qsb[:], start=True, stop=True)
            t1 = rope.tile([128, 512], F32, tag="t1", name="t1")
            nc.vector.tensor_mul(t1[:], qsb[:], ctc[c][:])
            t2 = rope.tile([128, 512], F32, tag="t2", name="t2")
            nc.vector.tensor_mul(t2[:], qsw[:], stc[c][:])
            nc.vector.tensor_add(dst[:], t1[:], t2[:])

        def kproj(c):
            ps = psP.tile([128, 512], F32, tag="psP", name="psk")
            for k in range(KT):
                nc.tensor.matmul(
                    ps[:], wk[:, k * HD:(k + 1) * HD],
                    xtc[c][:, k * 512:(k + 1) * 512],
                    start=(k == 0), stop=(k == KT - 1))
            rope_chunk(ps, bk_t[:, 0:1], kTc[c], c)

        def vproj(c):
            ps = psP.tile([128, 512], F32, tag="psP", name="psv")
            for k in range(KT):
                nc.tensor.matmul(
                    ps[:], wv[:, k * HD:(k + 1) * HD],
                    xtc[c][:, k * 512:(k + 1) * 512],
                    start=(k == 0), stop=(k == KT - 1))
            vsb = vsp.tile([128, 512], BF16, tag="vsb", name="vsb")
            nc.scalar.copy(vsb[:], ps[:])
            for tt in range(4):
                nc.sync.dma_start(
                    vA[4 * c + tt][:], vsb[:, tt * 128:(tt + 1) * 128],
                    transpose=True)

        def qproj(c, h):
            ps = psP.tile([128, 512], F32, tag="psP", name="psq")
            for k in range(KT):
                nc.tensor.matmul(
                    ps[:], wq[:, k * 512 + h * HD:k * 512 + (h + 1) * HD],
                    xtc[c][:, k * 512:(k + 1) * 512],
                    start=(k == 0), stop=(k == KT - 1))
            rope_chunk(ps, bq_t[:, h:h + 1], qTc[h][c], c)

        # ---- attention for one 512-query chunk -----------------------------
        otn = [None] * NQH

        def attention(qc):
            nfull = 4 * qc
            for h in range(NQH):
                psot = psOT.tile([128, 512], F32, tag="psot", name="psot")
                acc = accp.tile([128, 512], F32, tag="acc", name="acc")
                accb = accbp.tile([128, 512], BF16, tag="accb", name="accb")
                for tk in range(nfull):
                    pss = psS.tile([128, 512], F32, tag="pss", name="pss")
                    nc.tensor.matmul(
                        pss[:], kTc[tk // 4][:, (tk % 4) * 128:(tk % 4 + 1) * 128],
                        qTc[h][qc][:], start=True, stop=True)
                    pt = ptp.tile([128, 512], BF16, tag="pt", name="pt")
                    nc.scalar.activation(
                        pt[:], pss[:], mybir.ActivationFunctionType.Exp)
                    if tk == 0:
                        nc.vector.tensor_copy(acc[:], pt[:])
                    else:
                        nc.vector.tensor_add(acc[:], acc[:], pt[:])
                    nc.tensor.matmul(
                        psot[:], vA[tk][:], pt[:],
                        start=(tk == 0), stop=False)
                for j in range(4):
                    tk = nfull + j
                    sl = slice(j * 128, 512)
                    dsl = slice(j * 128, (j + 1) * 128)
                    pss = psS.tile([128, 512], F32, tag="pss", name="pss")
                    nc.tensor.matmul(
                        pss[:, sl], kTc[tk // 4][:, (tk % 4) * 128:(tk % 4 + 1) * 128],
                        qTc[h][qc][:, sl], start=True, stop=True)
                    pt = ptp.tile([128, 512], BF16, tag="pt", name="pt")
                    nc.scalar.activation(
                        pt[:, sl], pss[:, sl], mybir.ActivationFunctionType.Exp)
                    nc.vector.tensor_mul(pt[:, dsl], pt[:, dsl], tri[:])
                    if qc == 0 and j == 0:
                        nc.vector.tensor_copy(acc[:], pt[:])
                    elif j == 3:
                        nc.vector.tensor_add(accb[:, sl], acc[:, sl], pt[:, sl])
                        if j * 128 > 0:
                            nc.vector.tensor_copy(accb[:, 0:j * 128], acc[:, 0:j * 128])
                    else:
                        nc.vector.tensor_add(acc[:, sl], acc[:, sl], pt[:, sl])
                    nc.tensor.matmul(
                        psot[:, sl], vA[tk][:], pt[:, sl],
                        start=(qc == 0 and j == 0), stop=(j == 3))
                psb = psB.tile([128, 512], F32, tag="psb", name="psb")
                nc.tensor.matmul(psb[:], ones128[:], accb[:], start=True, stop=True)
                rinv = rinvp.tile([128, 512], F32, tag="rinv", name="rinv")
                nc.vector.reciprocal_approx_fast(out=rinv[:], in_=psb[:])
                ot = otp.tile([128, 512], BF16, tag="otn", name="ot")
                nc.vector.tensor_mul(ot[:], psot[:], rinv[:])
                otn[h] = ot

        def outproj(qc):
            for i in range(4):
                osb = outp.tile([128, E], F16, tag="osb", name="osb")
                for e in range(4):
                    psf = psP.tile([128, 512], F32, tag="psP", name="psf")
                    for h in range(NQH):
                        nc.tensor.matmul(
                            psf[:], otn[h][:, i * 128:(i + 1) * 128],
                            wo[:, h * E + e * 512:h * E + (e + 1) * 512],
                            start=(h == 0), stop=(h == NQH - 1))
                    if e % 2 == 0:
                        nc.vector.tensor_copy(osb[:, e * 512:(e + 1) * 512], psf[:])
                    else:
                        nc.scalar.copy(osb[:, e * 512:(e + 1) * 512], psf[:])
                nc.scalar.dma_start(
                    out_d[(qc * 4 + i) * 128:(qc * 4 + i + 1) * 128, :], osb[:])

        # ---- chunk-pipelined schedule --------------------------------------
        kproj(0)
        vproj(0)
        for h in range(NQH):
            qproj(0, h)
        for c in range(NC):
            attention(c)
            if c + 1 < NC:
                kproj(c + 1)
                vproj(c + 1)
            outproj(c)
            if c + 1 < NC:
                for h in range(NQH):
                    qproj(c + 1, h)
    nc.compile()
    return nc


def _rope_tables():
    # quirk: freq exponent uses full n_embed then slices to head_dim//2
    freqs = 10000.0 ** (-(np.arange(0, E, 2, dtype=np.float64) / E))[:HD // 2]
    t = np.arange(T, dtype=np.float64)
    ang = np.outer(freqs, t)                      # [64, T]
    ct = np.empty((HD, T), np.float32)
    st = np.empty((HD, T), np.float32)
    ct[:64] = np.cos(ang)
    ct[64:] = np.cos(ang)
    st[:64] = -np.sin(ang)
    st[64:] = np.sin(ang)
    return ct, st


def kernel(x, Wq, bq, Wk, bk, Wv, bv, Wo, bo):
    global _program, LAST_EXEC_NS, LAST_TRACE, LAST_PROFILE_JSON
    x = np.asarray(x, np.float32)
    Wq, bq = np.asarray(Wq, np.float32), np.asarray(bq, np.float32)
    Wk, bk = np.asarray(Wk, np.float32), np.asarray(bk, np.float32)
    Wv, bv = np.asarray(Wv, np.float32), np.asarray(bv, np.float32)
    Wo, bo = np.asarray(Wo, np.float32), np.asarray(bo, np.float32)
    bf = ml_dtypes.bfloat16

    if _program is None:
        _program = _build_program()

    perm = np.concatenate([np.arange(0, HD, 2), np.arange(1, HD, 2)])
    ct, st = _rope_tables()
    tri = (np.arange(128)[None, :] >= np.arange(128)[:, None]).astype(np.float32)
    psw = np.zeros((128, 128), np.float32)
    psw[(np.arange(128) + 64) % 128, np.arange(128)] = 1.0

    xT = [np.ascontiguousarray(x[b].T).astype(np.float16) for b in range(2)]
    in_maps = []
    for c in range(8):
        b, g = divmod(c, 4)
        qcols = np.concatenate([(4 * g + h) * HD + perm for h in range(NQH)])
        kcols = g * HD + perm
        vcols = np.arange(g * HD, (g + 1) * HD)
        in_maps.append({
            "xT": xT[b],
            "wq": Wq[:, qcols].astype(np.float16),
            "wk": Wk[:, kcols].astype(np.float16),
            "wv": Wv[:, vcols].astype(np.float16),
            "wo": (Wo[g * 512:(g + 1) * 512, :] * SCALE).astype(bf),
            "ct": ct,
            "st": st,
            "tri": tri.astype(bf),
            "psw": psw.astype(bf),
            "bq": np.ascontiguousarray(
                bq[np.concatenate([(4 * g + h) * HD + perm for h in range(NQH)])]
                .reshape(NQH, HD).T).astype(np.float32),
            "bk": bk[kcols].reshape(HD, 1).astype(np.float32),
        })

    import time
    t0 = time.time()
    res = run_bass_kernel_spmd(_program, in_maps, list(range(8)))
    t1 = time.time()
    LAST_EXEC_NS = res.exec_time_ns
    if res.instructions_and_trace is not None:
        LAST_TRACE = res.instructions_and_trace[1]
    LAST_PROFILE_JSON = res.profile_json
    if LAST_EXEC_NS is None:
        LAST_EXEC_NS = int((t1 - t0) * 1e9)  # wall time incl. H2D (upper bound)

    out = np.zeros((2, T, E), np.float64)
    for c in range(8):
        out[c // 4] += np.asarray(res.results[c]["out"], np.float64)
    # bv folded: after softmax each row sums to 1, scaled by SCALE inside Wo
    obias = np.repeat(bv.astype(np.float64).reshape(4, HD), 4, axis=0).reshape(-1)
    bo_eff = bo.astype(np.float64) + SCALE * (obias @ Wo.astype(np.float64))
    out += bo_eff[None, None, :]
    return out.astype(np.float32)


# revision 11
# speedup vs baseline: 1.0481x; 1.0481x over previous
"""GQA attention block (B=2,T=2048,E=2048,H=16,KV=4) on 8 trn2 NeuronCores.

Sharding: core c -> batch b=c//4, kv-group g=c%4 (q-heads 4g..4g+3, kv head g).
Each core computes its 4 heads end-to-end plus the partial output projection
(Wo rows for its heads); host sums the 4 partials per batch and adds bias.

v3 schedule/layout:
  - Chunk-pipelined: per 512-token chunk c, emit attention(qc=c), K/V
    projections of c+1, out-proj(c), Q projections of c+1 — PE never idles
    at phase boundaries, HAM stays warm.
  - RoPE half-swap via a permutation matmul on PE (no SBUF->SBUF DMA, which
    serialized against the vA transpose-DMAs).
  - Causal diagonal processed at 128-column granularity (skips the upper
    triangle's wasted matmul/exp work); single [128,128] triangle mask.
  - Softmax denominators: exp tiles accumulated in bf16 on DVE (2x mode),
    one all-ones[128,128] matmul broadcasts column sums to every partition,
    reciprocal_approx_fast + tensor_mul divides.
  - V projection computes V^T (wv stationary -> LDWEIGHTS hidden), XBAR
    transpose-DMA into [s,d] tiles for the PV matmul.
  - Output stored f16; host accumulates partials in f64.
"""

import numpy as np

for _p in ("/opt/trn_rl_repo", "/root/.axon_site/_ro/trn_rl_repo"):
    import sys

    if _p not in sys.path:
        sys.path.insert(0, _p)

import ml_dtypes
from contextlib import ExitStack

import concourse.bass as bass
import concourse.mybir as mybir
import concourse.tile as tile
from concourse import bacc
from concourse.bass_utils import run_bass_kernel_spmd

F32 = mybir.dt.float32
BF16 = mybir.dt.bfloat16
F16 = mybir.dt.float16
T = 2048
E = 2048
HD = 128
NQH = 4          # q heads per core
KT = E // 128    # 16 k-tiles over embed
NC = T // 512    # 4 512-chunks over time
SCALE = float(E) ** -0.5

_program = None
LAST_EXEC_NS = None
LAST_TRACE = None
LAST_PROFILE_JSON = None


def _build_program():
    nc = bacc.Bacc("TRN2", target_bir_lowering=False, debug=False, num_devices=8)
    xT_d = nc.declare_dram_parameter("xT", [E, T], F16, isOutput=False)
    wq_d = nc.declare_dram_parameter("wq", [E, NQH * HD], F16, isOutput=False)
    wk_d = nc.declare_dram_parameter("wk", [E, HD], F16, isOutput=False)
    wv_d = nc.declare_dram_parameter("wv", [E, HD], F16, isOutput=False)
    wo_d = nc.declare_dram_parameter("wo", [NQH * HD, E], BF16, isOutput=False)
    ct_d = nc.declare_dram_parameter("ct", [HD, T], F32, isOutput=False)
    st_d = nc.declare_dram_parameter("st", [HD, T], F32, isOutput=False)
    tri_d = nc.declare_dram_parameter("tri", [HD, HD], BF16, isOutput=False)
    psw_d = nc.declare_dram_parameter("psw", [HD, HD], BF16, isOutput=False)
    bq_d = nc.declare_dram_parameter("bq", [HD, NQH], F32, isOutput=False)
    bk_d = nc.declare_dram_parameter("bk", [HD, 1], F32, isOutput=False)
    out_d = nc.declare_dram_parameter("out", [T, E], F16, isOutput=True)

    with tile.TileContext(nc) as tc, ExitStack() as ctx:
        consts = ctx.enter_context(tc.tile_pool(name="consts", bufs=1))
        rope = ctx.enter_context(tc.tile_pool(name="rope", bufs=2))
        vsp = ctx.enter_context(tc.tile_pool(name="vsp", bufs=2))
        ptp = ctx.enter_context(tc.tile_pool(name="ptp", bufs=6))
        accp = ctx.enter_context(tc.tile_pool(name="accp", bufs=2))
        rinvp = ctx.enter_context(tc.tile_pool(name="rinvp", bufs=2))
        otp = ctx.enter_context(tc.tile_pool(name="otp", bufs=6))
        outp = ctx.enter_context(tc.tile_pool(name="outp", bufs=2))
        psP = ctx.enter_context(tc.tile_pool(name="psP", bufs=2, space=bass.MemorySpace.PSUM))
        psS = ctx.enter_context(tc.tile_pool(name="psS", bufs=2, space=bass.MemorySpace.PSUM))
        psW = ctx.enter_context(tc.tile_pool(name="psW", bufs=1, space=bass.MemorySpace.PSUM))
        psOT = ctx.enter_context(tc.tile_pool(name="psOT", bufs=2, space=bass.MemorySpace.PSUM))
        psB = ctx.enter_context(tc.tile_pool(name="psB", bufs=1, space=bass.MemorySpace.PSUM))

        # ---- persistent tiles ---------------------------------------------
        wk = consts.tile([128, KT * HD], F16, tag="wk", name="wk")
        wv = consts.tile([128, KT * HD], F16, tag="wv", name="wv")
        wq = consts.tile([128, KT * NQH * HD], F16, tag="wq", name="wq")
        wo = consts.tile([128, NQH * E], BF16, tag="wo", name="wo")
        xtc = [consts.tile([128, KT * 512], F16, tag=f"xtc{c}", name=f"xtc{c}")
               for c in range(NC)]
        ctc = [consts.tile([128, 512], F32, tag=f"ctc{c}", name=f"ctc{c}")
               for c in range(NC)]
        stc = [consts.tile([128, 512], F32, tag=f"stc{c}", name=f"stc{c}")
               for c in range(NC)]
        tri = consts.tile([128, 128], BF16, tag="tri", name="tri")
        psw = consts.tile([128, 128], BF16, tag="psw", name="psw")
        bq_t = consts.tile([HD, NQH], F32, tag="bq", name="bq_t")
        bk_t = consts.tile([HD, 1], F32, tag="bk", name="bk_t")
        ones128 = consts.tile([128, 128], BF16, tag="ones", name="ones128")
        nc.vector.memset(ones128[:], 1.0)

        qTc = [[consts.tile([128, 512], BF16, tag=f"qT{h}_{c}", name=f"qT{h}_{c}")
                for c in range(NC)] for h in range(NQH)]
        kTc = [consts.tile([128, 512], BF16, tag=f"kT{c}", name=f"kT{c}")
               for c in range(NC)]
        vA = [consts.tile([128, 128], BF16, tag=f"vA{t}", name=f"vA{t}")
              for t in range(4 * NC)]

        def split_rows(src_ap, p=128):
            # [(k p), f] -> [p, k, f]: one DMA that deposits each 128-row
            # band k into its own column block of the destination tile.
            return src_ap.rearrange("(k p) f -> p k f", p=p)

        # ---- input DMA issue order (chunk-major so compute starts early) --
        nc.sync.dma_start(wk[:], split_rows(wk_d[:, :]))
        nc.sync.dma_start(bk_t[:], bk_d[:, :])
        nc.sync.dma_start(bq_t[:], bq_d[:, :])
        nc.sync.dma_start(psw[:], psw_d[:, :])
        nc.sync.dma_start(tri[:], tri_d[:, :])
        nc.sync.dma_start(xtc[0][:], split_rows(xT_d[:, 0:512]))
        nc.sync.dma_start(wv[:], split_rows(wv_d[:, :]))
        nc.sync.dma_start(ctc[0][:], ct_d[:, 0:512])
        nc.sync.dma_start(stc[0][:], st_d[:, 0:512])
        nc.sync.dma_start(wq[:], split_rows(wq_d[:, :]))
        for c in range(1, NC):
            nc.sync.dma_start(xtc[c][:], split_rows(xT_d[:, c * 512:(c + 1) * 512]))
            nc.sync.dma_start(ctc[c][:], ct_d[:, c * 512:(c + 1) * 512])
            nc.sync.dma_start(stc[c][:], st_d[:, c * 512:(c + 1) * 512])
            if c == 2:
                nc.sync.dma_start(wo[:], split_rows(wo_d[:, :]))

        # ---- projections ---------------------------------------------------
        def rope_chunk(ps, bias_ap, dst, c):
            qsb = rope.tile([128, 512], BF16, tag="qsb", name="qsb")
            nc.scalar.activation(
                qsb[:], ps[:], mybir.ActivationFunctionType.Identity, bias=bias_ap)
            qsw = psW.tile([128, 512], F32, tag="qsw", name="qsw")
            nc.tensor.matmul(qsw[:], psw[:], qsb[:], start=True, stop=True)
            t1 = rope.tile([128, 512], F32, tag="t1", name="t1")
            nc.vector.tensor_mul(t1[:], qsb[:], ctc[c][:])
            t2 = rope.tile([128, 512], F32, tag="t2", name="t2")
            nc.vector.tensor_mul(t2[:], qsw[:], stc[c][:])
            nc.vector.tensor_add(dst[:], t1[:], t2[:])

        def kproj(c):
            ps = psP.tile([128, 512], F32, tag="psP", name="psk")
            for k in range(KT):
                nc.tensor.matmul(
                    ps[:], wk[:, k * HD:(k + 1) * HD],
                    xtc[c][:, k * 512:(k + 1) * 512],
                    start=(k == 0), stop=(k == KT - 1))
            rope_chunk(ps, bk_t[:, 0:1], kTc[c], c)

        def vproj(c):
            ps = psP.tile([128, 512], F32, tag="psP", name="psv")
            for k in range(KT):
                nc.tensor.matmul(
                    ps[:], wv[:, k * HD:(k + 1) * HD],
                    xtc[c][:, k * 512:(k + 1) * 512],
                    start=(k == 0), stop=(k == KT - 1))
            vsb = vsp.tile([128, 512], BF16, tag="vsb", name="vsb")
            nc.scalar.copy(vsb[:], ps[:])
            for tt in range(4):
                nc.sync.dma_start(
                    vA[4 * c + tt][:], vsb[:, tt * 128:(tt + 1) * 128],
                    transpose=True)

        def qproj(c, h):
            ps = psP.tile([128, 512], F32, tag="psP", name="psq")
            for k in range(KT):
                nc.tensor.matmul(
                    ps[:], wq[:, k * 512 + h * HD:k * 512 + (h + 1) * HD],
                    xtc[c][:, k * 512:(k + 1) * 512],
                    start=(k == 0), stop=(k == KT - 1))
            rope_chunk(ps, bq_t[:, h:h + 1], qTc[h][c], c)

        # ---- attention for one 512-query chunk -----------------------------
        otn = [None] * NQH

        def attention(qc):
            nfull = 4 * qc
            for h in range(NQH):
                psot = psOT.tile([128, 512], F32, tag="psot", name="psot")
                acc = accp.tile([128, 512], BF16, tag="acc", name="acc")
                for tk in range(nfull):
                    pss = psS.tile([128, 512], F32, tag="pss", name="pss")
                    nc.tensor.matmul(
                        pss[:], kTc[tk // 4][:, (tk % 4) * 128:(tk % 4 + 1) * 128],
                        qTc[h][qc][:], start=True, stop=True)
                    pt = ptp.tile([128, 512], BF16, tag="pt", name="pt")
                    nc.scalar.activation(
                        pt[:], pss[:], mybir.ActivationFunctionType.Exp)
                    if tk == 0:
                        nc.vector.tensor_copy(acc[:], pt[:])
                    else:
                        nc.vector.tensor_add(acc[:], acc[:], pt[:])
                    nc.tensor.matmul(
                        psot[:], vA[tk][:], pt[:],
                        start=(tk == 0), stop=False)
                for j in range(4):
                    tk = nfull + j
                    sl = slice(j * 128, 512)
                    dsl = slice(j * 128, (j + 1) * 128)
                    pss = psS.tile([128, 512], F32, tag="pss", name="pss")
                    nc.tensor.matmul(
                        pss[:, sl], kTc[tk // 4][:, (tk % 4) * 128:(tk % 4 + 1) * 128],
                        qTc[h][qc][:, sl], start=True, stop=True)
                    pt = ptp.tile([128, 512], BF16, tag="pt", name="pt")
                    nc.scalar.activation(
                        pt[:, sl], pss[:, sl], mybir.ActivationFunctionType.Exp)
                    nc.vector.tensor_mul(pt[:, dsl], pt[:, dsl], tri[:])
                    if qc == 0 and j == 0:
                        nc.vector.tensor_copy(acc[:], pt[:])
                    else:
                        nc.vector.tensor_add(acc[:, sl], acc[:, sl], pt[:, sl])
                    nc.tensor.matmul(
                        psot[:, sl], vA[tk][:], pt[:, sl],
                        start=(qc == 0 and j == 0), stop=(j == 3))
                psb = psB.tile([128, 512], F32, tag="psb", name="psb")
                nc.tensor.matmul(psb[:], ones128[:], acc[:], start=True, stop=True)
                rinv = rinvp.tile([128, 512], F32, tag="rinv", name="rinv")
                nc.vector.reciprocal_approx_fast(out=rinv[:], in_=psb[:])
                ot = otp.tile([128, 512], BF16, tag="otn", name="ot")
                nc.vector.tensor_mul(ot[:], psot[:], rinv[:])
                otn[h] = ot

        def outproj(qc):
            for i in range(4):
                osb = outp.tile([128, E], F16, tag="osb", name="osb")
                for e in range(4):
                    psf = psP.tile([128, 512], F32, tag="psP", name="psf")
                    for h in range(NQH):
                        nc.tensor.matmul(
                            psf[:], otn[h][:, i * 128:(i + 1) * 128],
                            wo[:, h * E + e * 512:h * E + (e + 1) * 512],
                            start=(h == 0), stop=(h == NQH - 1))
                    if e % 2 == 0:
                        nc.vector.tensor_copy(osb[:, e * 512:(e + 1) * 512], psf[:])
                    else:
                        nc.scalar.copy(osb[:, e * 512:(e + 1) * 512], psf[:])
                nc.sync.dma_start(
                    out_d[(qc * 4 + i) * 128:(qc * 4 + i + 1) * 128, :], osb[:])

        # ---- chunk-pipelined schedule --------------------------------------
        kproj(0)
        vproj(0)
        for h in range(NQH):
            qproj(0, h)
        for c in range(NC):
            attention(c)
            if c + 1 < NC:
                kproj(c + 1)
                vproj(c + 1)
            outproj(c)
            if c + 1 < NC:
                for h in range(NQH):
                    qproj(c + 1, h)
    nc.compile()
    return nc


def _rope_tables():
    # quirk: freq exponent uses full n_embed then slices to head_dim//2
    freqs = 10000.0 ** (-(np.arange(0, E, 2, dtype=np.float64) / E))[:HD // 2]
    t = np.arange(T, dtype=np.float64)
    ang = np.outer(freqs, t)                      # [64, T]
    ct = np.empty((HD, T), np.float32)
    st = np.empty((HD, T), np.float32)
    ct[:64] = np.cos(ang)
    ct[64:] = np.cos(ang)
    st[:64] = -np.sin(ang)
    st[64:] = np.sin(ang)
    return ct, st


def kernel(x, Wq, bq, Wk, bk, Wv, bv, Wo, bo):
    global _program, LAST_EXEC_NS, LAST_TRACE, LAST_PROFILE_JSON
    x = np.asarray(x, np.float32)
    Wq, bq = np.asarray(Wq, np.float32), np.asarray(bq, np.float32)
    Wk, bk = np.asarray(Wk, np.float32), np.asarray(bk, np.float32)
    Wv, bv = np.asarray(Wv, np.float32), np.asarray(bv, np.float32)
    Wo, bo = np.asarray(Wo, np.float32), np.asarray(bo, np.float32)
    bf = ml_dtypes.bfloat16

    if _program is None:
        _program = _build_program()

    perm = np.concatenate([np.arange(0, HD, 2), np.arange(1, HD, 2)])
    ct, st = _rope_tables()
    tri = (np.arange(128)[None, :] >= np.arange(128)[:, None]).astype(np.float32)
    psw = np.zeros((128, 128), np.float32)
    psw[(np.arange(128) + 64) % 128, np.arange(128)] = 1.0

    xT = [np.ascontiguousarray(x[b].T).astype(np.float16) for b in range(2)]
    in_maps = []
    for c in range(8):
        b, g = divmod(c, 4)
        qcols = np.concatenate([(4 * g + h) * HD + perm for h in range(NQH)])
        kcols = g * HD + perm
        vcols = np.arange(g * HD, (g + 1) * HD)
        in_maps.append({
            "xT": xT[b],
            "wq": Wq[:, qcols].astype(np.float16),
            "wk": Wk[:, kcols].astype(np.float16),
            "wv": Wv[:, vcols].astype(np.float16),
            "wo": (Wo[g * 512:(g + 1) * 512, :] * SCALE).astype(bf),
            "ct": ct,
            "st": st,
            "tri": tri.astype(bf),
            "psw": psw.astype(bf),
            "bq": np.ascontiguousarray(
                bq[np.concatenate([(4 * g + h) * HD + perm for h in range(NQH)])]
                .reshape(NQH, HD).T).astype(np.float32),
            "bk": bk[kcols].reshape(HD, 1).astype(np.float32),
        })

    import time
    t0 = time.time()
    res = run_bass_kernel_spmd(_program, in_maps, list(range(8)))
    t1 = time.time()
    LAST_EXEC_NS = res.exec_time_ns
    if res.instructions_and_trace is not None:
        LAST_TRACE = res.instructions_and_trace[1]
    LAST_PROFILE_JSON = res.profile_json
    if LAST_EXEC_NS is None:
        LAST_EXEC_NS = int((t1 - t0) * 1e9)  # wall time incl. H2D (upper bound)

    out = np.zeros((2, T, E), np.float64)
    for c in range(8):
        out[c // 4] += np.asarray(res.results[c]["out"], np.float64)
    # bv folded: after softmax each row sums to 1, scaled by SCALE inside Wo
    obias = np.repeat(bv.astype(np.float64).reshape(4, HD), 4, axis=0).reshape(-1)
    bo_eff = bo.astype(np.float64) + SCALE * (obias @ Wo.astype(np.float64))
    out += bo_eff[None, None, :]
    return out.astype(np.float32)
